# revision 28
# baseline (speedup 1.0000x reference)
"""Trainium2 Bass kernel for nn_CostFn_18562848653837.

reference(x, cond, time) only reads x[b, j, 6+k] for j in [0,26), k in [0,6)
(~2.6 MB of the 436 MB input; cond/time are unused) and computes, per point,
the reflected mass 1 / (u^T J M^{-1} J^T u) with u = e_x, which reduces via
Sherman-Morrison (M = 2I + 0.5 c c^T, c = cos(cq), s = sin(cq), v = L*s,
cq = cumsum(q)) to

    denom = 0.5*||v||^2 - 0.125*(c.v)^2 / (1 + 0.25*||c||^2)

and further, with double-angle identities, to pure functions of sin^2(cq)
and sin(2*cq):

    ||v||^2 = sum_k L_k^2 sin^2(cq_k)            =: Q1
    c.v     = 0.5 * sum_k L_k sin(2 cq_k)        =  0.5 * P2
    ||c||^2 = 6 - sum_k sin^2(cq_k)              =  6 - Q3
    denom   = 0.5*Q1 - 0.03125*P2^2 / (2.5 - 0.25*Q3)

Both sin^2(th) and sin(2 th) are invariant under th -> th - k*pi for any
integer k, so range reduction mod pi needs no off-by-one fixup. The mod is
done with the magic-number trick (adding/subtracting 1.5*2^23 rounds to
nearest integer in pure f32 adds); the ACT engine's Sin (valid domain
[-pi, pi]) then gets |m| <= pi/2 and |2m| <= pi. No clamp is emitted: the
DVE/Pool ALUs are IEEE f32, so the reduced angles are bit-deterministic for
the fixed input set and were verified in-range in CoreSim.

Work is spread over engines: cumsum + critical-slice range reduction + WA +
Q1 on DVE, remaining range-reduction slices + WS + P2 + Q3 on GpSimd (Pool),
Sin/2-angle-Sin + small affine ops on ACT, final partition reduction on PE.

Sharding: pure data parallel over batch - core i gets batches
[512*i, 512*(i+1)), i.e. 512*26 = 13312 points laid out as a (128, 104) tile
per q-component. Each core emits one f32 partial sum; host adds the 8.
"""

import numpy as np

_P, _W, _K = 128, 104, 6
_F = _K * _W
_NCORES = 8
_B, _H, _T = 4096, 1024, 26
_BPC = _B // _NCORES  # batches per core

_CACHE = {}


def _get_nc():
    if "nc" in _CACHE:
        return _CACHE["nc"]

    import concourse.tile as tile
    import concourse.mybir as mybir
    from concourse import bacc

    PI = float(np.pi)
    PI32 = float(np.float32(np.pi))
    EIGHT_PI = float(8.0 * np.pi)
    INV_PI = 1.0 / PI
    MAGIC = 12582912.0  # 1.5 * 2^23: f32 add/sub rounds to nearest int
    L = [float(np.float32(v)) for v in np.arange(1, 7) * 0.1 + 0.3]

    f32 = mybir.dt.float32
    AX = mybir.AxisListType
    OP = mybir.AluOpType
    ACT = mybir.ActivationFunctionType

    nc = bacc.Bacc(
        "TRN2", target_bir_lowering=False, debug=False, num_devices=_NCORES
    )
    q_dram = nc.dram_tensor("q", [_K, _P, _W], f32, kind="ExternalInput")
    out_dram = nc.dram_tensor("out", [_P, 1], f32, kind="ExternalOutput")

    with (
        tile.TileContext(nc) as tc,
        tc.tile_pool(name="pool", bufs=1) as pool,
    ):
        # Dep-free dummy Sin on the pre-initialized const-1.0 AP: the Sin
        # table-set load is hoisted before ACT's first Sin, and by making
        # that first Sin dependency-free the ~1.3us load runs at t~0,
        # hidden behind the input DMAs instead of stalling the real Sin.
        one_ap = nc.const_aps.aps[(f32, 1.0)]
        WARM = pool.tile([_P, 1], f32)
        nc.scalar.activation(WARM[:], one_ap[:_P], ACT.Sin)

        # one tile per q-plane so the cumsum can chase the DMAs; split the
        # issues across the two DMA-capable sequencers (500 ns issue each)
        Qk = []
        for k in range(_K):
            qk = pool.tile([_P, _W], f32, tag=f"q{k}")
            eng = nc.sync if k % 2 == 0 else nc.gpsimd
            eng.dma_start(qk[:], q_dram[k])
            Qk.append(qk)

        # cq_k = q_0 + ... + q_k (DVE), with +8*pi seeded into block 0 so
        # the magic-number argument is positive (8*pi is a multiple of pi,
        # to which the double-angle quantities are invariant)
        CQ = pool.tile([_P, _F], f32)
        nc.vector.tensor_scalar(CQ[:, 0:_W], Qk[0][:], EIGHT_PI, None, OP.add)
        for k in range(1, _K):
            nc.vector.tensor_add(
                CQ[:, k * _W : (k + 1) * _W],
                CQ[:, (k - 1) * _W : k * _W],
                Qk[k][:],
            )

        # range reduction m = cq - pi*round(cq/pi), pipelined behind the
        # cumsum: batched slices k={0,1,2} and k={3,4} on Pool, the critical
        # last plane on DVE right after its cumsum add. Pool has no
        # scalar_tensor_tensor (walrus engine check), so fold the -pi
        # multiply into the magic subtract: PK = (u - MAGIC) * -pi, then
        # m = PK + cq.
        U = pool.tile([_P, _F], f32)
        PK = pool.tile([_P, _F], f32)
        RC = pool.tile([_P, _F], f32)
        for k in range(_K):
            sl = slice(k * _W, (k + 1) * _W)
            eng = nc.vector if k == _K - 1 else nc.gpsimd
            eng.tensor_scalar(U[:, sl], CQ[:, sl], INV_PI, MAGIC, OP.mult, OP.add)
            eng.tensor_scalar(PK[:, sl], U[:, sl], MAGIC, -PI32, OP.subtract, OP.mult)
            eng.tensor_add(RC[:, sl], PK[:, sl], CQ[:, sl])

        # ACT: sin(m), then sin(2m) = sin(2cq); DVE squares sin(m) in
        # parallel with the second Sin
        SM = pool.tile([_P, _F], f32)
        nc.scalar.activation(SM[:], RC[:], ACT.Sin)
        SMSQ = pool.tile([_P, _F], f32)
        nc.vector.tensor_mul(SMSQ[:], SM[:], SM[:])
        SF = pool.tile([_P, _F], f32)
        nc.scalar.activation(SF[:], RC[:], ACT.Sin, scale=2.0)

        # weighted planes: WA = L^2 * sin^2 (DVE), WS = L * sin2 (Pool)
        WA = pool.tile([_P, _F], f32)
        WS = pool.tile([_P, _F], f32)
        for k in range(_K):
            sl = slice(k * _W, (k + 1) * _W)
            nc.vector.tensor_scalar_mul(WA[:, sl], SMSQ[:, sl], L[k] * L[k])
        for k in range(_K):
            sl = slice(k * _W, (k + 1) * _W)
            nc.gpsimd.tensor_scalar_mul(WS[:, sl], SF[:, sl], L[k])

        # block sums over k -> (128, 104): Q3/TC/P2 on Pool, Q1 on DVE
        Q1 = pool.tile([_P, _W], f32)
        P2 = pool.tile([_P, _W], f32)
        Q3 = pool.tile([_P, _W], f32)
        nc.gpsimd.tensor_add(Q3[:], SMSQ[:, 0:_W], SMSQ[:, _W : 2 * _W])
        for k in range(2, _K):
            nc.gpsimd.tensor_add(Q3[:], Q3[:], SMSQ[:, k * _W : (k + 1) * _W])
        TC = pool.tile([_P, _W], f32)
        nc.gpsimd.tensor_scalar(TC[:], Q3[:], -0.25, 2.5, OP.mult, OP.add)
        nc.gpsimd.tensor_add(P2[:], WS[:, 0:_W], WS[:, _W : 2 * _W])
        for k in range(2, _K):
            nc.gpsimd.tensor_add(P2[:], P2[:], WS[:, k * _W : (k + 1) * _W])
        nc.vector.reduce_sum(
            Q1[:], WA[:].rearrange("p (k w) -> p w k", k=_K), axis=AX.X
        )

        # denom = 0.5*Q1 - 0.03125*P2^2 / TC with TC = 2.5 - 0.25*Q3.
        # Multiply through by TC to avoid a second reciprocal:
        #   cost = TC / (0.5*Q1*TC - 0.03125*P2^2)   (TC in [1, 2.5] > 0)
        G = pool.tile([_P, _W], f32)
        nc.vector.scalar_tensor_tensor(G[:], Q1[:], 0.5, TC[:], OP.mult, OP.mult)
        TB = pool.tile([_P, _W], f32)
        nc.vector.scalar_tensor_tensor(TB[:], P2[:], 0.03125, P2[:], OP.mult, OP.mult)
        D = pool.tile([_P, _W], f32)
        nc.vector.tensor_sub(D[:], G[:], TB[:])
        WREC = pool.tile([_P, _W], f32)
        nc.vector.reciprocal(WREC[:], D[:])
        COST = pool.tile([_P, _W], f32)
        nc.vector.tensor_mul(COST[:], TC[:], WREC[:])

        colsum = pool.tile([_P, 1], f32)
        nc.vector.reduce_sum(colsum[:], COST[:], axis=AX.X)
        nc.sync.dma_start(out_dram[:], colsum[:])

    nc.compile()
    _CACHE["nc"] = nc
    return nc


def _shard(x):
    qs = np.ascontiguousarray(x[:, :_T, 6 : 6 + _K], dtype=np.float32)
    return np.ascontiguousarray(
        qs.reshape(_NCORES, _BPC * _T, _K).transpose(0, 2, 1).reshape(
            _NCORES, _K, _P, _W
        )
    )


def _get_runner():
    """Build the jitted 8-core shard_map executable once (mirrors
    bass2jax.run_bass_via_pjrt's multi-core path) so repeat kernel() calls
    skip retracing/recompiling."""
    if "run" in _CACHE:
        return _CACHE["run"]
    import jax
    from jax.sharding import Mesh, PartitionSpec
    from jax.experimental.shard_map import shard_map
    from concourse import bass2jax

    nc = _get_nc()
    bass2jax.install_neuronx_cc_hook()
    assert nc.dbg_addr is None
    pid_name = nc.partition_id_tensor.name if nc.partition_id_tensor else None
    in_names = ("q", "out") + ((pid_name,) if pid_name else ())

    out_aval = jax.core.ShapedArray((_P, 1), np.float32)

    def _body(q, out_zero):
        operands = [q, out_zero]
        if pid_name is not None:
            operands.append(bass2jax.partition_id_tensor())
        (out,) = bass2jax._bass_exec_p.bind(
            *operands,
            out_avals=(out_aval,),
            in_names=in_names,
            out_names=("out",),
            lowering_input_output_aliases=(),
            sim_require_finite=True,
            sim_require_nnan=True,
            nc=nc,
        )
        return (out,)

    devices = jax.devices()[:_NCORES]
    mesh = Mesh(np.asarray(devices), ("core",))
    sharded = jax.jit(
        shard_map(
            _body,
            mesh=mesh,
            in_specs=(PartitionSpec("core"),) * 2,
            out_specs=(PartitionSpec("core"),),
            check_rep=False,
        ),
        donate_argnums=(1,),
        keep_unused=True,
    )

    def run(planes):
        concat_q = planes.reshape(_NCORES * _K, _P, _W)
        zeros = np.zeros((_NCORES * _P, 1), np.float32)
        (out,) = sharded(concat_q, zeros)
        return np.asarray(out)  # (8*128, 1)

    _CACHE["run"] = run
    return run


def kernel(x, cond, time):
    x = np.asarray(x)
    planes = _shard(x)
    try:
        run = _get_runner()
        partials = run(planes).astype(np.float32)
    except Exception:
        # fall back to the library SPMD runner if the cached fast path
        # breaks (e.g. jax API drift)
        from concourse.bass_utils import run_bass_kernel_spmd

        res = run_bass_kernel_spmd(
            _get_nc(),
            [{"q": planes[i]} for i in range(_NCORES)],
            list(range(_NCORES)),
        )
        partials = np.stack([r["out"][:, 0] for r in res.results]).astype(
            np.float32
        )
    return np.float32(partials.sum(dtype=np.float32))
